# revision 32
# baseline (speedup 1.0000x reference)
"""Trainium2 Bass kernel for DCMLayer: 1x1 conv -> per-sample dynamic 3x3
depthwise conv -> 1x1 fuse conv, data-parallel over 8 NeuronCores.

Contract: kernel(**inputs) takes the FULL unsharded inputs
(x[32,256,96,96], conv_w[64,256], conv_b[64], dw_b[64], fuse_w[256,64],
fuse_b[256]) and returns the full y[32,256,96,96] float32.

Per-core layout: 4 samples as 2 two-sample packs. Within a pack, sample a
occupies partitions 0:64 and sample b 64:128; all matmuls touching
per-sample data use square diagonal PE tiles (64,64)@(0,0) / @(64,64).
mm1 runs in float32r directly on the loaded x tiles (no fp16 conversion
pass). The per-sample dynamic 3x3 depthwise conv runs on the PE as 9
diagonal fp16 matmuls accumulating in PSUM over a zero-padded flat f
layout (98-wide rows), ordered tap-major over 4-chunk groups so weight
loads amortize. g = pool(conv(x)) + b is computed as pool(mm1 psum)/1024
+ b by linearity, so x is read once.

DMA layout: every load/store spans all 128 SBUF partitions in a single
dma_start (both samples of a pack at once; stores carry all 4 output
channel chunks) so all 16 SBUF AXI ports are active per transfer.
"""
import numpy as np

import concourse.bacc as bacc
import concourse.bass as bass
import concourse.tile as tile
from concourse import mybir
from concourse.bass_utils import run_bass_kernel_spmd

F32 = mybir.dt.float32
F32R = mybir.dt.float32r
F16 = mybir.dt.float16
AF = mybir.ActivationFunctionType
ALU = mybir.AluOpType
AX = mybir.AxisListType

# Problem geometry (hardcoded per contract)
N, C, H, W = 32, 256, 96, 96
Cm, P = 64, 256
HW = H * W           # 9216
NCORES = 8
NLOC = N // NCORES   # 4 samples per core
NPACK = NLOC // 2    # 2 two-sample packs per core
KC4 = C // 64        # 4 K=64 contraction chunks for mm1
MC4 = P // 64        # 4 M=64 output chunks for mm2

WP = W + 2           # padded row width 98
FPAD = WP * (H + 2) + 2  # padded f buffer 9606 (+2 slack for corner taps)
RS = 4               # rows per compute chunk
NCH = H // RS        # 24 chunks per pack
NT = RS * W          # 384 = compute tile free size
NDW = RS * WP        # 392 dw output positions per chunk
LR = 8               # rows per x load strip
NLD = H // LR        # 12 load strips
SUBS = LR // RS      # 2 mm1 sub-chunks per strip (processed in pairs)
GR = 12              # rows per y store group
NG = H // GR         # 8 store groups
QG = GR // RS        # 3 chunks per store group
BR = 32              # pooling block rows/cols

_CACHED = {}


def build_nc():
    nc = bacc.Bacc("TRN2", target_bir_lowering=False, debug=False)

    # x pre-packed on host: [pack, cc, (s*64+c), HW]; y returned packed as
    # [pack, mc, (s*64+c), HW] and unpacked on host.
    x_d = nc.dram_tensor("x", [NPACK, KC4, 128, HW], F32R,
                         kind="ExternalInput").ap()
    cwB_d = nc.dram_tensor("cwB", [128, KC4 * 128], F32R, kind="ExternalInput").ap()
    fwB_d = nc.dram_tensor("fwB", [128, MC4 * 128], F16, kind="ExternalInput").ap()
    cb2_d = nc.dram_tensor("cb2", [128, 1], F32, kind="ExternalInput").ap()
    fba_d = nc.dram_tensor("fba", [128, MC4], F32, kind="ExternalInput").ap()
    id_d = nc.dram_tensor("ident", [128, 128], F16, kind="ExternalInput").ap()
    y_d = nc.dram_tensor("y", [NPACK, MC4, 128, HW], F32,
                         kind="ExternalOutput").ap()

    with tile.TileContext(nc) as tc:
        build_body(nc, tc, x_d, cwB_d, fwB_d, cb2_d, fba_d, id_d, y_d)
    nc.compile()
    return nc


def build_body(nc, tc, x_d, cwB_d, fwB_d, cb2_d, fba_d, id_d, y_d):
    ctxs = []

    def pool(**kw):
        p = tc.tile_pool(**kw)
        ctxs.append(p)
        return p.__enter__()

    consts = pool(name="consts", bufs=1)
    xpool = pool(name="xs", bufs=6)
    fpads = pool(name="fpads", bufs=1)
    opool = pool(name="osb", bufs=6)
    ypool = pool(name="ysb", bufs=3)
    small = pool(name="small", bufs=1)
    diagp = pool(name="diagp", bufs=1)
    psA = pool(name="psA", bufs=2, space="PSUM")
    psD = pool(name="psD", bufs=3, space="PSUM")
    psY = pool(name="psY", bufs=3, space="PSUM")

    # ---- constants ----
    cwB = consts.tile([128, KC4 * 128], F32R)  # block-diag conv_w^T chunks
    nc.sync.dma_start(cwB[:], cwB_d)
    fwB = consts.tile([128, MC4 * 128], F16)   # block-diag fuse_w^T chunks
    nc.sync.dma_start(fwB[:], fwB_d)
    cb2 = consts.tile([128, 1], F32)
    nc.sync.dma_start(cb2[:], cb2_d)
    fba = consts.tile([128, MC4], F32)
    nc.sync.dma_start(fba[:], fba_d)
    ident = consts.tile([128, 128], F16)
    nc.sync.dma_start(ident[:], id_d)

    fpad = [fpads.tile([128, FPAD], F16, tag=f"fpad{pk}", name=f"fpad{pk}")
            for pk in range(NPACK)]
    for pk in range(NPACK):
        nc.gpsimd.memset(fpad[pk][:], 0.0)

    xparts = [small.tile([128, NCH * 3], F32, tag=f"xp{pk}", name=f"xp{pk}")
              for pk in range(NPACK)]
    diag9 = [diagp.tile([128, 9 * 128], F16, tag=f"d{pk}", name=f"diag9{pk}")
             for pk in range(NPACK)]

    def phaseA_strip(pk, ld):
        r0 = ld * LR
        xt = xpool.tile([128, KC4 * LR * W], F32R, tag="xt", name="xt")
        # one DMA per strip spanning all 128 partitions / 16 ports
        nc.sync.dma_start(
            xt[:].rearrange("p (cc f) -> p cc f", cc=KC4),
            x_d[pk, :, :, r0 * W:(r0 + LR) * W].rearrange(
                "cc p f -> p cc f"))
        # mm1 in fp32r straight off the loaded tile; sub-chunks in pairs,
        # cc-major within a pair so the PE streams 2 per weight load
        for half in range(SUBS // 2):
            pAs = [psA.tile([128, NT], F32, tag="pA", name="pA")
                   for _ in range(2)]
            for cc in range(KC4):
                for j in range(2):
                    sub = half * 2 + j
                    nc.tensor.matmul(
                        pAs[j][:],
                        cwB[:, cc * 128:(cc + 1) * 128],
                        xt[:, cc * LR * W + sub * NT:
                           cc * LR * W + (sub + 1) * NT],
                        start=(cc == 0), stop=(cc == KC4 - 1),
                    )
            for j in range(2):
                sub = half * 2 + j
                ch = ld * SUBS + sub
                rr = r0 + sub * RS
                # f evict: relu(psum + conv_b) -> fpad fp16, 98-wide rows
                base = (rr + 1) * WP + 1
                dst = fpad[pk][:, base:base + RS * WP].rearrange(
                    "p (r w) -> p r w", w=WP)[:, :, 0:W]
                nc.scalar.activation(dst, pAs[j][:], AF.Relu, bias=cb2[:])
                # pooling partial sums (pre-relu, pre-bias)
                pv = pAs[j][:].rearrange("p (r cb w) -> p cb r w",
                                         r=RS, cb=3, w=BR)
                nc.vector.tensor_reduce(
                    xparts[pk][:, ch * 3:(ch + 1) * 3], pv,
                    axis=AX.XY, op=ALU.add)

    SRB = NCH // 3       # 8 chunks per 32-row pooling block

    def phaseA_final_rb(pk, rb):
        # dynamic-kernel taps for pooling row-block rb (taps rb*3..rb*3+2):
        # ready as soon as this row-block's strips are done
        xp3 = small.tile([128, 3], F32, tag=f"xp3_{pk}_{rb}",
                         name=f"xp3_{pk}_{rb}")
        nc.vector.tensor_reduce(
            xp3[:],
            xparts[pk][:, rb * 3 * SRB:(rb + 1) * 3 * SRB].rearrange(
                "p (s cb) -> p cb s", cb=3),
            axis=AX.X, op=ALU.add)
        g3 = small.tile([128, 3], F32, tag=f"g3_{pk}_{rb}",
                        name=f"g3_{pk}_{rb}")
        nc.vector.tensor_scalar(
            out=g3[:], in0=xp3[:], scalar1=1.0 / (BR * BR), scalar2=cb2[:],
            op0=ALU.mult, op1=ALU.add)
        for cb in range(3):
            t = rb * 3 + cb
            dst = diag9[pk][:, t * 128:(t + 1) * 128]
            if cb % 2 == 0:
                nc.vector.tensor_scalar_mul(dst, ident[:], g3[:, cb:cb + 1])
            else:
                nc.scalar.activation(dst, ident[:], AF.Identity,
                                     scale=g3[:, cb:cb + 1])

    def phaseB_group(pk, gi):
        ysb = ypool.tile([128, MC4 * GR * W], F32, tag="ysb", name="ysb")
        # dw: tap-major over the group's 4 chunks -> 9 weight loads/group
        pDs = [psD.tile([128, NDW], F32, tag="pD", name="pD")
               for _ in range(QG)]
        ti = 0
        for dy in (-1, 0, 1):
            for dx in (-1, 0, 1):
                for q in range(QG):
                    rr = (gi * QG + q) * RS
                    off = (rr + 1) * WP + 1 + dy * WP + dx
                    nc.tensor.matmul(
                        pDs[q][:], diag9[pk][:, ti * 128:(ti + 1) * 128],
                        fpad[pk][:, off:off + NDW],
                        start=(ti == 0), stop=(ti == 8),
                    )
                ti += 1
        osbs = []
        for q in range(QG):
            osb = opool.tile([128, NT], F16, tag="osb", name="osb")
            src = pDs[q][:, 0:RS * WP].rearrange(
                "p (r w) -> p r w", w=WP)[:, :, 0:W]
            nc.scalar.copy(osb[:], src)
            osbs.append(osb)
        # mm2: mc-major so one weight load covers the group's 4 chunks
        for mc in range(MC4):
            for q in range(QG):
                pY = psY.tile([128, NT], F32, tag="pY", name="pY")
                nc.tensor.matmul(
                    pY[:], fwB[:, mc * 128:(mc + 1) * 128], osbs[q][:],
                    start=True, stop=True,
                )
                dst = ysb[:, (mc * GR + q * RS) * W:
                          (mc * GR + q * RS) * W + NT]
                if (mc + q) % 2 == 0:
                    nc.vector.tensor_scalar_add(dst, pY[:], fba[:, mc:mc + 1])
                else:
                    nc.scalar.activation(dst, pY[:], AF.Identity,
                                         bias=fba[:, mc:mc + 1])
        return (pk, gi, ysb)

    def flush_store(item):
        pk, gi, ysb = item
        # one DMA per group on the ACT HWDGE ring: stores never block the
        # load ring (qSPDynamicHW) at the sequencer
        nc.scalar.dma_start(
            y_d[pk, :, :, gi * GR * W:(gi + 1) * GR * W].rearrange(
                "mc p f -> p mc f"),
            ysb[:].rearrange("p (mc f) -> p mc f", mc=MC4))

    def run_b(pk, gi):
        flush_store(phaseB_group(pk, gi))

    # software pipeline: A(0); [A(pk+1) interleaved with B(pk)]; B(last)
    for ld in range(NLD):
        phaseA_strip(0, ld)
        if ld % 4 == 3:
            phaseA_final_rb(0, ld // 4)
    for pk in range(NPACK):
        if pk + 1 < NPACK:
            # 12 strips spread over 8 groups: 2,1,2,1,...; a row-block's
            # diag taps are emitted as soon as its 4 strips are done
            ld = 0
            for gi in range(NG):
                for _ in range(2 if gi % 2 == 0 else 1):
                    phaseA_strip(pk + 1, ld)
                    if ld % 4 == 3:
                        phaseA_final_rb(pk + 1, ld // 4)
                    ld += 1
                run_b(pk, gi)
        else:
            for gi in range(NG):
                run_b(pk, gi)

    for p in reversed(ctxs):
        p.__exit__(None, None, None)


def _prep(inputs):
    x = np.ascontiguousarray(inputs["x"], dtype=np.float32)
    conv_w = np.asarray(inputs["conv_w"], dtype=np.float32)
    conv_b = np.asarray(inputs["conv_b"], dtype=np.float32)
    dw_b = np.asarray(inputs["dw_b"], dtype=np.float32)
    fuse_w = np.asarray(inputs["fuse_w"], dtype=np.float32)
    fuse_b = np.asarray(inputs["fuse_b"], dtype=np.float32)

    cwT = np.ascontiguousarray(conv_w.T)                      # [256, 64]
    cwB = np.zeros((128, KC4 * 128), np.float32)
    for cc in range(KC4):
        blk = cwT[cc * 64:(cc + 1) * 64, :]                   # [64 k, 64 m]
        cwB[0:64, cc * 128:cc * 128 + 64] = blk
        cwB[64:128, cc * 128 + 64:(cc + 1) * 128] = blk
    fwT = np.ascontiguousarray(fuse_w.T)                      # [64, 256]
    fwB = np.zeros((128, MC4 * 128), np.float16)
    for mc in range(MC4):
        blk = fwT[:, mc * 64:(mc + 1) * 64]                   # [64 k, 64 m]
        fwB[0:64, mc * 128:mc * 128 + 64] = blk
        fwB[64:128, mc * 128 + 64:(mc + 1) * 128] = blk
    cb2 = np.tile(conv_b, 2)[:, None].astype(np.float32)      # [128, 1]
    fba_flat = (fuse_b + fuse_w @ dw_b).astype(np.float32)    # [256]
    fba = np.stack([np.tile(fba_flat[mc * 64:(mc + 1) * 64], 2)
                    for mc in range(MC4)], axis=1)            # [128, 4]
    ident = np.eye(128, dtype=np.float16)

    # pack x to [core, pack, cc, (s*64+c), HW]: pure permutation, f32
    xr = x.reshape(NCORES, NPACK, 2, KC4, 64, HW)
    xr = np.ascontiguousarray(xr.transpose(0, 1, 3, 2, 4, 5)).reshape(
        NCORES, NPACK, KC4, 128, HW)
    in_maps = []
    for i in range(NCORES):
        in_maps.append({
            "x": xr[i],
            "cwB": cwB,
            "fwB": fwB,
            "cb2": cb2,
            "fba": fba,
            "ident": ident,
        })
    return in_maps


def run(inputs, trace=False, tmpdir=None):
    if "nc" not in _CACHED:
        _CACHED["nc"] = build_nc()
    nc = _CACHED["nc"]
    in_maps = _prep(inputs)
    res = run_bass_kernel_spmd(nc, in_maps, list(range(NCORES)), trace=trace,
                               tmpdir=tmpdir)
    # y arrives packed [pack, mc, (s*64+c), HW]; unpack to [N, P, H, W]
    yp = np.stack([res.results[i]["y"] for i in range(NCORES)], axis=0)
    yp = yp.reshape(NCORES, NPACK, MC4, 2, 64, HW).transpose(0, 1, 3, 2, 4, 5)
    return np.ascontiguousarray(yp).reshape(N, P, H, W), res


def kernel(**inputs):
    y, _ = run(inputs, trace=False)
    return y
